# revision 9
# baseline (speedup 1.0000x reference)
"""Trainium2 Bass kernel for nn_CNN_CharEmb.

Computation: character embeddings -> pointwise conv (per-position linear) ->
ragged per-word max-pool over the 7 chars of each word:

  out[b, w, :] = max_{k=0..6} ( emb[x[b, 8w+k]] @ conv_w.T + conv_b )

Device strategy (8 NeuronCores, batch-sharded, 4 rows/core):
  1. Fused table M' = emb @ conv_w.T + conv_b  [72, 300] bf16 built on
     device by one matmul (a ones-row in emb^T paired with a bias-row in
     conv_w^T folds the bias into the contraction), so h[pos] = M'[x[pos]]
     and embedding+conv collapse into a row-select.
  2. The row-select is a one-hot matmul: onehot [72, L] bf16 (a pure
     re-encoding of x, built host-side like the index tensors) makes
     h_k tile = onehot_slice.T @ M' a PE matmul.
  3. Per word-tile (128 words), 7 matmuls (char slots k=0..6) land in
     PSUM banks 0..6 of a single 8-bank pool; the fold is balanced over
     the two engines that can read PSUM: ACT batch-copies banks 0..4 to
     SBUF bf16 (two ops, releasing banks early), DVE max-folds banks
     5..6 against them in place and finishes the bf16 max tree.
     Output is written bf16 and upcast on the host (well within the
     2e-2 tolerance).

`wordidx` is the fixed 7-chars+boundary pattern of the reference setup;
anything else falls back to an exact host computation.
"""

import numpy as np
import ml_dtypes

import concourse.bacc as bacc
import concourse.mybir as mybir
import concourse.tile as tile
from concourse import bass_utils

# Problem shape (hardcoded per contract)
B = 32
WORD_LEN = 7
NUM_WORDS = 400
STRIDE = WORD_LEN + 1            # 8
L = NUM_WORDS * STRIDE           # 3200
EMB = 100
OUT = 300
VOCAB = 70

N_CORES = 8
B_CORE = B // N_CORES            # 4 batch rows per core
NW = B_CORE * NUM_WORDS          # 1600 words per core
LC = B_CORE * L                  # 12800 positions per core
N_TILES = (NW + 127) // 128      # 13 word-tiles (last one 64 words)
KDIM = EMB + 1                   # 101: emb + ones/bias row
VPAD = 72                        # vocab padded to 72 (DMA 44% lighter than 128)

BF16 = mybir.dt.bfloat16
F32 = mybir.dt.float32

LAST_RESULTS = None  # stashed BassKernelResults for the test harness


def _build_program():
    nc = bacc.Bacc("TRN2", target_bir_lowering=False, debug=False,
                   num_devices=N_CORES)

    oh_dram = nc.dram_tensor("oh", [VPAD, LC], BF16, kind="ExternalInput")
    # embT_aug [101, 72] and wt_aug [101, 300] packed side by side: one DMA.
    const_dram = nc.dram_tensor("consts", [KDIM, VPAD + OUT], BF16,
                                kind="ExternalInput")
    out_dram = nc.dram_tensor("out", [NW, OUT], BF16, kind="ExternalOutput")

    with tile.TileContext(nc) as tc:
        with (
            tc.tile_pool(name="const", bufs=1) as cpool,
            tc.tile_pool(name="oh", bufs=1) as ohpool,
            tc.tile_pool(name="res", bufs=1) as rpool,
            tc.tile_pool(name="work", bufs=3) as wpool,
            tc.tile_pool(name="ps", bufs=1, space="PSUM") as ppool,
        ):
            const_t = cpool.tile([KDIM, VPAD + OUT], BF16)
            oh = ohpool.tile([VPAD, LC], BF16)
            oh3 = oh[:].rearrange("p (w k) -> p w k", k=STRIDE)
            nc.sync.dma_start(const_t[:], const_dram[:])

            # Fused table M' = emb @ W.T + b  [72, 300] bf16, built in the
            # spare 8th PSUM bank.  Issued BEFORE the one-hot chunk loads so
            # its DMA-semaphore wait doesn't get lumped behind them.
            p_pre = ppool.tile([128, 1, 512], F32, tag="sp")
            nc.tensor.matmul(p_pre[0:VPAD, 0, 0:OUT],
                             const_t[:, 0:VPAD], const_t[:, VPAD:],
                             start=True, stop=True)
            mprime = cpool.tile([VPAD, OUT], BF16)
            nc.scalar.copy(mprime[:], p_pre[0:VPAD, 0, 0:OUT])

            # host-built one-hot, loaded in chunks (first chunk gates tile 0)
            TILE_P = 128 * STRIDE                      # 1024 positions
            bounds = [0, 2 * TILE_P, 5 * TILE_P, 9 * TILE_P, LC]
            for c0, c1 in zip(bounds, bounds[1:]):
                nc.sync.dma_start(oh[:, c0:c1], oh_dram[:, c0:c1])

            RES = rpool.tile([128, N_TILES, OUT], BF16)

            for t in range(N_TILES):
                rows = min(128, NW - t * 128)
                w0, w1 = t * 128, t * 128 + rows
                # Per-bank-group PSUM tiles so banks free individually
                # (a whole-tile allocation would stall tile t+1's matmuls
                # on ALL of tile t's drains).
                PA = ppool.tile([128, 2, 512], F32, tag="pa")  # k0,k1
                PB = ppool.tile([128, 3, 512], F32, tag="pb")  # k2,k3,k4
                PC = ppool.tile([128, 2, 512], F32, tag="pc")  # k5,k6
                for k in range(2):
                    nc.tensor.matmul(PA[0:rows, k, 0:OUT],
                                     oh3[0:VPAD, w0:w1, k], mprime[:],
                                     start=True, stop=True)
                for k in range(3):
                    nc.tensor.matmul(PB[0:rows, k, 0:OUT],
                                     oh3[0:VPAD, w0:w1, 2 + k], mprime[:],
                                     start=True, stop=True)
                for k in range(2):
                    nc.tensor.matmul(PC[0:rows, k, 0:OUT],
                                     oh3[0:VPAD, w0:w1, 5 + k], mprime[:],
                                     start=True, stop=True)

                # Two-engine fold (only ACT and DVE can read PSUM):
                #   ACT: W[3:5]=copy(k0,k1)   W[0:3]=copy(k2,k3,k4)
                #   DVE: W[3:5]=max([k5,k6], W[3:5]) -> m05, m16
                #        W[2:4]=max([k2,k3], [k4,m05]) -> m24, m035
                #        W[3]  =max(m24, m035)
                #        res   =max(W[3], m16)
                W = wpool.tile([128, 5, OUT], BF16, tag="W")
                nc.scalar.copy(W[0:rows, 3:5, :], PA[0:rows, 0:2, 0:OUT])
                nc.scalar.copy(W[0:rows, 0:3, :], PB[0:rows, 0:3, 0:OUT])
                nc.vector.tensor_max(W[0:rows, 3:5, :], PC[0:rows, 0:2, 0:OUT],
                                     W[0:rows, 3:5, :])
                nc.vector.tensor_max(W[0:rows, 2:4, :], W[0:rows, 0:2, :],
                                     W[0:rows, 2:4, :])
                nc.vector.tensor_max(W[0:rows, 3, :], W[0:rows, 2, :],
                                     W[0:rows, 3, :])
                nc.vector.tensor_max(RES[0:rows, t, :], W[0:rows, 3, :],
                                     W[0:rows, 4, :])

            # Output stores: 4 batched DMAs (bf16)
            for t0, t1 in ((0, 4), (4, 8), (8, 12)):
                view = out_dram[t0 * 128:t1 * 128, :].rearrange(
                    "(t p) o -> p t o", p=128)
                nc.sync.dma_start(view, RES[:, t0:t1, :])
            nc.sync.dma_start(out_dram[1536:1600, :], RES[0:64, 12, :])

    nc.compile()
    return nc


def _host_inputs(x, emb_table, conv_w, conv_b):
    """Build per-core device input tensors (layout/dtype prep only)."""
    bf16 = ml_dtypes.bfloat16

    consts = np.zeros((KDIM, VPAD + OUT), bf16)
    consts[:EMB, :VOCAB] = emb_table.T.astype(bf16)
    consts[EMB, :VOCAB] = bf16(1.0)                  # ones row -> bias
    consts[:EMB, VPAD:] = conv_w.T.astype(bf16)
    consts[EMB, VPAD:] = conv_b.astype(bf16)

    ohs = []
    vv = np.arange(VPAD)[:, None]
    for c in range(N_CORES):
        xc = x[c * B_CORE:(c + 1) * B_CORE].reshape(-1)   # [12800]
        ohs.append((xc[None, :] == vv).astype(bf16))

    return consts, ohs


def _expected_wordidx():
    pattern = np.concatenate([np.ones(WORD_LEN, np.int64), np.zeros(1, np.int64)])
    return np.tile(pattern, NUM_WORDS)[None, :].repeat(B, axis=0)


def _host_fallback(x, wordidx, emb_table, conv_w, conv_b):
    """Exact reference math on host (only for unexpected wordidx layouts)."""
    e = emb_table[x]
    h = np.einsum('blc,oc->blo', e, conv_w) + conv_b
    bi = (wordidx == 0).astype(np.int64)
    word_id = np.cumsum(bi, axis=1) - bi
    word_id = np.minimum(word_id, NUM_WORDS - 1)
    valid = wordidx > 0
    out = np.full((B, NUM_WORDS, OUT), -np.inf, np.float32)
    for b in range(B):
        for w in range(NUM_WORDS):
            m = valid[b] & (word_id[b] == w)
            if m.any():
                out[b, w] = h[b, m].max(axis=0)
    return out


def kernel(x, wordidx, emb_table, conv_w, conv_b):
    global LAST_RESULTS
    x = np.asarray(x)
    wordidx = np.asarray(wordidx)
    emb_table = np.asarray(emb_table, np.float32)
    conv_w = np.asarray(conv_w, np.float32)
    conv_b = np.asarray(conv_b, np.float32)

    if not np.array_equal(wordidx.astype(np.int64), _expected_wordidx()):
        return _host_fallback(x.astype(np.int64), wordidx.astype(np.int64),
                              emb_table, conv_w, conv_b)

    consts, ohs = _host_inputs(
        x.astype(np.int64), emb_table, conv_w, conv_b)

    nc = _build_program()
    in_maps = [
        {"oh": ohs[c], "consts": consts}
        for c in range(N_CORES)
    ]
    res = bass_utils.run_bass_kernel_spmd(nc, in_maps,
                                          core_ids=list(range(N_CORES)))
    LAST_RESULTS = res
    out = np.concatenate([res.results[c]["out"] for c in range(N_CORES)], axis=0)
    return out.reshape(B, NUM_WORDS, OUT).astype(np.float32)
